# revision 23
# baseline (speedup 1.0000x reference)
"""Trainium2 Bass kernel for BayesLinear sampling forward (v5h: bf16 stream).

Math (per sample b):
    out[b,o] = sum_i (eps_w[b,o,i] * exp(weight_psi)[o,i] + weight_mu[o,i]) * x[b,i]
             + eps_b[b,o] * exp(bias_psi)[o] + bias_mu[o]

Data-parallel over batch B=1024 across 8 cores (128 samples each).

v5 structure (the f32 baseline was HBM-bound at ~423us; streaming eps_w
as bf16 halves HBM bytes -> ~192us floor):
  - Host pre-transposes eps_w per core to [b, p, ic, o] bf16 (i = ic*128+p),
    giving 4KB-contiguous runs per partition (4KB descriptors measured ~7%
    faster per byte than 8KB ones). x/psi/mu also host-cast to bf16.
  - eps DMAs all on the sync HWDGE ring: the scalar (ACT) queue runs the
    csb copies, and a dma_start queued behind a copy starves the ring.
  - DVE: z_b = epsT_b * ET elementwise (bf16 tensor_tensor, 2x packed
    mode), one op per sample, FD=2048.
  - PE: C_b[b', o] = sum_i x[b', i] * z_b[i, o] via lhsT = xT chunks
    (bf16, [128 i, 128 b']); 4 ic-chunk matmuls accumulate in PSUM.
    Row b' == b is exactly the eps-term for sample b.
  - ACT evacuates C (PSUM->SBUF bf16, full-width: partition-sliced PSUM
    reads are illegal); a same-partition 1KB DMA pulls row b into G
    (SWDGE mid-stream, scalar HWDGE for the last pairs to cut drain
    latency).
  - All setup transposes on PE during the ramp (DMA-transpose measured
    ~6us per block -- far too slow); only the tiny cmu/f0 ops are emitted
    mid-loop, so the tail is one add + store.
"""

import sys

sys.path.insert(0, "/opt/trn_rl_repo")

import numpy as np

B, IN, OUT = 1024, 512, 512
NCORES = 8
BL = B // NCORES  # 128 samples per core
NPAIR = BL // 2

_CACHE = {}

EPS_BUFS = 8
PRE_ENQ = 6
HWDGE_EVAC_FROM = NPAIR - 8


def build():
    from contextlib import ExitStack

    import concourse.bacc as bacc
    import concourse.mybir as mybir
    import concourse.tile as tile

    f32 = mybir.dt.float32
    bf16 = mybir.dt.bfloat16
    Alu = mybir.AluOpType
    Act = mybir.ActivationFunctionType

    nc = bacc.Bacc("TRN2", target_bir_lowering=False, debug=False)

    x_d = nc.dram_tensor("x", [BL, IN], bf16, kind="ExternalInput").ap()
    epsw_d = nc.dram_tensor(
        "eps_w", [BL, 128, 4, OUT], bf16, kind="ExternalInput"
    ).ap()
    epsb_d = nc.dram_tensor("eps_b", [BL, OUT], f32, kind="ExternalInput").ap()
    wmu_d = nc.dram_tensor("weight_mu", [OUT, IN], bf16, kind="ExternalInput").ap()
    wpsi_d = nc.dram_tensor("weight_psi", [OUT, IN], bf16, kind="ExternalInput").ap()
    bmu_d = nc.dram_tensor("bias_mu", [1, OUT], f32, kind="ExternalInput").ap()
    bpsi_d = nc.dram_tensor("bias_psi", [1, OUT], f32, kind="ExternalInput").ap()
    id_d = nc.dram_tensor("ident", [128, 128], bf16, kind="ExternalInput").ap()
    out_d = nc.dram_tensor("out", [BL, OUT], f32, kind="ExternalOutput").ap()

    with tile.TileContext(nc) as tc, ExitStack() as ctx:
        perm = ctx.enter_context(tc.tile_pool(name="perm", bufs=1))
        strm = ctx.enter_context(tc.tile_pool(name="strm", bufs=4))

        def eps_dma(p):
            e = strm.tile(
                [128, 2, 4, OUT], bf16, tag="eps", bufs=EPS_BUFS, name=f"eps_{p}"
            )
            nc.sync.dma_start(
                e[:],
                epsw_d[2 * p : 2 * p + 2].rearrange("b p ic o -> p b ic o"),
            )
            return e

        # deep pre-enqueue on the sync ring before any setup load
        eps_tiles = [eps_dma(p) for p in range(PRE_ENQ)]

        # ---- setup loads (scalar ring) + PE transposes ----
        ident16 = perm.tile([128, 128], bf16)
        nc.scalar.dma_start(ident16[:], id_d)
        x_sb = perm.tile([128, IN], bf16)
        nc.scalar.dma_start(x_sb[:], x_d)

        # ET[p, ic, o] = exp(psi)[o, ic*128+p]  (bf16)
        ET = perm.tile([128, 4, OUT], bf16)
        muT = perm.tile([128, 4, OUT], bf16)
        xT16 = [perm.tile([128, 128], bf16, name=f"xT{i}") for i in range(4)]
        G = perm.tile([128, OUT], bf16)
        f0 = perm.tile([128, OUT], f32)

        with tc.tile_pool(name="pss", bufs=4, space="PSUM") as pss:
            for ic in range(4):
                tmp = pss.tile([128, 128], bf16, tag="pst")
                nc.tensor.transpose(
                    tmp[:], x_sb[:, ic * 128 : (ic + 1) * 128], ident16[:]
                )
                nc.scalar.copy(xT16[ic][:], tmp[:])
            for t in range(4):
                psi_sb = strm.tile([128, IN], bf16, tag="setup_ld")
                nc.scalar.dma_start(psi_sb[:], wpsi_d[t * 128 : (t + 1) * 128, :])
                for ic in range(4):
                    tmp = pss.tile([128, 128], bf16, tag="pst")
                    nc.tensor.transpose(
                        tmp[:], psi_sb[:, ic * 128 : (ic + 1) * 128], ident16[:]
                    )
                    nc.scalar.activation(
                        ET[:, ic, t * 128 : (t + 1) * 128], tmp[:], Act.Exp
                    )
            # mu transposes also at setup: PE sits idle until the first
            # stream matmul (~14us), so these are free here, and doing them
            # mid-loop stalled the PE FIFO behind the transpose ping-pong
            for t in range(4):
                mu_sb = strm.tile([128, IN], bf16, tag="setup_ld")
                nc.scalar.dma_start(mu_sb[:], wmu_d[t * 128 : (t + 1) * 128, :])
                for ic in range(4):
                    tmp = pss.tile([128, 128], bf16, tag="pst")
                    nc.tensor.transpose(
                        tmp[:], mu_sb[:, ic * 128 : (ic + 1) * 128], ident16[:]
                    )
                    nc.scalar.copy(muT[:, ic, t * 128 : (t + 1) * 128], tmp[:])

        # tail-only small loads + PE warm-up burst (HAM clock gate)
        epsb_sb = perm.tile([128, OUT], f32)
        nc.scalar.dma_start(epsb_sb[:], epsb_d)
        brow = perm.tile([1, OUT], f32)
        nc.scalar.dma_start(brow[:], bmu_d)
        prow = perm.tile([1, OUT], f32)
        nc.scalar.dma_start(prow[:], bpsi_d)
        erow = perm.tile([1, OUT], f32)
        nc.scalar.activation(erow[:], prow[:], Act.Exp)
        ones1 = perm.tile([1, 128], f32)
        nc.vector.memset(ones1[:], 1.0)
        for _ in range(24):
            nc.tensor.ldweights(ident16[:])

        with tc.tile_pool(name="psm", bufs=1, space="PSUM") as psm:
            ebias_bc = psm.tile([128, OUT], f32, tag="aux1")
            nc.tensor.matmul(ebias_bc[:], ones1[:], erow[:], start=True, stop=True)

            for p in range(NPAIR):
                et = eps_tiles[p] if p < PRE_ENQ else eps_dma(p)
                z = strm.tile(
                    [128, 2, 4, OUT], bf16, tag="z", bufs=3, name=f"z_{p}"
                )
                C = psm.tile([128, 2, OUT], f32, tag="C", bufs=3, name=f"C_{p}")
                for s in range(2):
                    nc.vector.tensor_tensor(z[:, s], et[:, s], ET[:], Alu.mult)
                for s in range(2):
                    for ic in range(4):
                        nc.tensor.matmul(
                            C[:, s, :],
                            xT16[ic][:],
                            z[:, s, ic, :],
                            start=(ic == 0),
                            stop=(ic == 3),
                        )
                # PSUM partition-sliced reads are illegal, so evacuate the
                # full tile (ACT cost is free-dim driven), then pull row b
                # (sample b's result) out with a same-partition 1KB DMA.
                csb = strm.tile(
                    [128, 2, OUT], bf16, tag="csb", bufs=3, name=f"csb_{p}"
                )
                nc.scalar.copy(csb[:], C[:])
                evac = nc.scalar if p >= HWDGE_EVAC_FROM else nc.gpsimd
                for s in range(2):
                    b = 2 * p + s
                    evac.dma_start(G[b : b + 1, :], csb[b : b + 1, s, :])

                # tiny mu/bias/f0 prep mid-loop (tail would cost ~14us)
                if p == 2:
                    nc.vector.tensor_mul(f0[:], epsb_sb[:], ebias_bc[:])
                if p == 4:
                    cmu = psm.tile([128, OUT], f32, tag="aux1")
                    for ic in range(4):
                        nc.tensor.matmul(
                            cmu[:],
                            xT16[ic][:],
                            muT[:, ic, :],
                            start=(ic == 0),
                            stop=False,
                        )
                    nc.tensor.matmul(
                        cmu[:], ones1[:], brow[:], start=False, stop=True
                    )
                if p == 6:
                    nc.vector.tensor_add(f0[:], f0[:], cmu[:])

        # ---- tail: combine eps-term rows with f0, store ----
        nc.vector.tensor_add(f0[:], f0[:], G[:])
        nc.sync.dma_start(out_d, f0[:])

    nc.compile()
    return nc


def _in_maps(x, eps_w, eps_b, weight_mu, weight_psi, bias_mu, bias_psi):
    import concourse.mybir as mybir

    bf16 = mybir.dt.np(mybir.dt.bfloat16)
    ident = np.eye(128, dtype=np.float32).astype(bf16)
    maps = []
    for c in range(NCORES):
        sl = slice(c * BL, (c + 1) * BL)
        # [b, o, i] -> [b, p, ic, o] with i = ic*128 + p (one fused permute+cast)
        ew = eps_w[sl].reshape(BL, OUT, 4, 128).transpose(0, 3, 2, 1)
        ew = np.ascontiguousarray(ew, dtype=bf16)
        maps.append(
            {
                "x": x[sl].astype(bf16),
                "eps_w": ew,
                "eps_b": np.ascontiguousarray(eps_b[sl], dtype=np.float32),
                "weight_mu": weight_mu.astype(bf16),
                "weight_psi": weight_psi.astype(bf16),
                "bias_mu": np.ascontiguousarray(
                    bias_mu.reshape(1, OUT), dtype=np.float32
                ),
                "bias_psi": np.ascontiguousarray(
                    bias_psi.reshape(1, OUT), dtype=np.float32
                ),
                "ident": ident,
            }
        )
    return maps


def kernel(x, eps_w, eps_b, weight_mu, weight_psi, bias_mu, bias_psi, **run_kwargs):
    from concourse.bass_utils import run_bass_kernel_spmd

    if "nc" not in _CACHE:
        _CACHE["nc"] = build()
    nc = _CACHE["nc"]
    maps = _in_maps(x, eps_w, eps_b, weight_mu, weight_psi, bias_mu, bias_psi)
    res = run_bass_kernel_spmd(nc, maps, list(range(NCORES)), **run_kwargs)
    out = np.concatenate([r["out"] for r in res.results], axis=0)
    _CACHE["last_results"] = res
    return out
